# revision 8
# baseline (speedup 1.0000x reference)
"""Trainium2 Bass kernel for nn_ConditionedEpsilonTheta.

Strategy:
- Data-parallel over batch: B=16 -> 2 batch elements per core x 8 cores.
- Host-side weight folding:
    q^T = (Wq @ q_W / sqrt(hd)) @ x            (query_proj fused into Q proj)
    k^T = (Wk @ dyn_W) @ dyn^T                 (dyn encoder fused into K proj)
    v   = dyn @ (Wv @ dyn_W)^T                 (dyn encoder fused into V proj)
  All biases folded / pre-laid-out on host.
- All matmuls in bf16 (1 cyc/row on PE, FWL weight loads). PSUM fp32.
- Attention in "transposed" layout: scoresT[j, i] so that exp(scoresT) feeds
  the ctx matmul directly as the moving operand (no attention transposes).
- Causal structure: query i attends dyn j <= i, so for i-tile [0,512) only
  j-tiles 0..3 are computed; for [512,1024) j-tiles 0..7. Fused rel-pos-bias +
  -1e9 mask is added into PSUM scores via a bf16 identity matmul. exp -> 0 for
  masked entries.
- Softmax denominator: v is stored with a ones-column per head (65-wide head
  slots); ctx matmul row 64 accumulates sum_j exp(s). Reciprocal via Ln then
  Exp(scale=-1) (same ACT table set), broadcast across partitions with a DMA
  through a DRAM bounce buffer.
"""

import os
import sys
import numpy as np
from contextlib import ExitStack

if "/opt/trn_rl_repo" not in sys.path:
    sys.path.insert(0, "/opt/trn_rl_repo")

import ml_dtypes

B, C, H, S, E, NH, DD, DS = 16, 256, 1024, 1024, 512, 8, 512, 64
HD = E // NH
REL_MAX = 2048
S1 = S + 1
NCORES = 8
BL = B // NCORES          # batch per core = 2
KC_C = C // 128           # 2
KE = E // 128             # 4
NJ = S // 128             # 8
NI = H // 512             # 2
NEG = np.float32(-30.0)

BF16 = ml_dtypes.bfloat16


# ----------------------------------------------------------------- host prep

def _host_prep(inputs):
    """Fold/transpose weights, build mask; returns per-core input maps."""
    f64 = lambda a: np.asarray(a, np.float64)
    x = np.asarray(inputs["x"], np.float32)
    dyn = np.asarray(inputs["dyn"], np.float32)
    static = np.asarray(inputs["static"], np.float32)

    in_W = f64(inputs["in_proj_W"])
    in_b = f64(inputs["in_proj_b"])
    Wq, Wk, Wv = in_W[:E], in_W[E:2 * E], in_W[2 * E:]
    bq, bk, bv = in_b[:E], in_b[E:2 * E], in_b[2 * E:]
    q_W, q_b = f64(inputs["q_W"]), f64(inputs["q_b"])
    dyn_W, dyn_b = f64(inputs["dyn_W"]), f64(inputs["dyn_b"])
    sc = 1.0 / np.sqrt(HD)

    shared = {}
    bf = lambda a: np.ascontiguousarray(np.asarray(a, np.float32), dtype=BF16)
    f32 = lambda a: np.ascontiguousarray(a, dtype=np.float32)

    shared["wq"] = bf(((Wq @ q_W) * sc).T)               # [C, E]
    shared["bqv"] = f32((Wq @ q_b + bq) * sc)            # [E]
    shared["wk"] = bf((Wk @ dyn_W).T)                    # [DD, E]
    shared["bkv"] = f32(Wk @ dyn_b + bk)                 # [E]
    shared["wv"] = bf((Wv @ dyn_W).T)                    # [DD, E]
    shared["bvv"] = f32(Wv @ dyn_b + bv)                 # [E] (row bias)
    shared["wkr"] = bf(Wk.T)                             # [E, E]
    shared["bkrv"] = f32(bk)
    shared["wvr"] = bf(Wv.T)                             # [E, E]
    shared["bvrv"] = f32(bv)
    shared["wst"] = bf(f64(inputs["stat_W"]).T)          # [DS, E]
    shared["bstv"] = f32(f64(inputs["stat_b"]))
    shared["wf1"] = bf(f64(inputs["film_W1"]).T)         # [E, E]
    shared["bf1v"] = f32(f64(inputs["film_b1"]))
    shared["wf2"] = bf(f64(inputs["film_W2"]).T)         # [E, C]
    shared["bf2v"] = f32(f64(inputs["film_b2"]))
    shared["wout"] = bf(f64(inputs["out_W"]).T)          # [E, E]
    shared["wctx"] = bf(f64(inputs["ctx_W"]).T)          # [E, C]
    shared["cbv"] = f32(f64(inputs["ctx_W"]) @ f64(inputs["out_b"])
                        + f64(inputs["ctx_b"]))          # [C]

    # fused rel-pos bias + causal mask, transposed: maskT[j, i], j in [0,S1)
    tbl = np.asarray(inputs["bias_table"], np.float32)
    i_ids = np.arange(H)[None, :]
    j_ids = np.arange(S1)[:, None]
    rel = np.clip(i_ids - j_ids, -REL_MAX + 1, REL_MAX - 1)
    maskT = tbl[rel + REL_MAX - 1].astype(np.float32)
    blocked = (j_ids < S) & (j_ids > i_ids)
    maskT = np.where(blocked, NEG, maskT)
    shared["maskt"] = bf(maskT)                          # [S1, H]
    shared["ident"] = bf(np.eye(128))
    shared["onesv"] = bf(np.ones((128, NH * 65)))
    shared["ones8"] = bf(np.ones((1, NH)))

    per_core = []
    for c in range(NCORES):
        lo = c * BL
        m = dict(shared)
        m["x"] = bf(x[lo:lo + BL])                                  # [BL,C,H]
        m["xf"] = f32(x[lo:lo + BL])                                # exact x
        m["dynt"] = bf(dyn[lo:lo + BL].transpose(0, 2, 1))          # [BL,DD,S]
        m["statt"] = bf(static[lo:lo + BL].T)                       # [DS, BL]
        per_core.append(m)
    return per_core


# ------------------------------------------------------------- bass program

def _emit(ctx: ExitStack, tc, aps):
    import concourse.bass as bass
    from concourse import mybir

    nc = tc.nc
    F32 = mybir.dt.float32
    BF = mybir.dt.bfloat16
    AF = mybir.ActivationFunctionType

    # ---- pools
    wp = ctx.enter_context(tc.tile_pool(name="wp", bufs=1))
    ab = ctx.enter_context(tc.tile_pool(name="ab", bufs=1))      # per-batch
    sc1 = ctx.enter_context(tc.tile_pool(name="sc1", bufs=4))    # dynT/outT
    ep = ctx.enter_context(tc.tile_pool(name="ep", bufs=4))      # E tiles
    sp = ctx.enter_context(tc.tile_pool(name="sp", bufs=3))      # small rows
    rp = ctx.enter_context(tc.tile_pool(name="rp", bufs=2))      # recip bcast
    op = ctx.enter_context(tc.tile_pool(name="op", bufs=2))      # out f32
    pp_s = ctx.enter_context(tc.tile_pool(name="pps", bufs=2, space="PSUM"))
    pp_c = ctx.enter_context(tc.tile_pool(name="ppc", bufs=2, space="PSUM"))
    dp = ctx.enter_context(tc.tile_pool(name="dp", bufs=4, space="DRAM"))

    def loadw(name, rows, cols, dt=BF):
        """DRAM [rows, cols] -> list of [128, cols] SBUF tiles."""
        n = (rows + 127) // 128
        out = []
        for k in range(n):
            r = min(128, rows - k * 128)
            t = wp.tile([r, cols], dt, name=f"{name}_{k}", tag=f"{name}_{k}")
            nc.sync.dma_start(t[:], aps[name][k * 128:k * 128 + r, :])
            out.append(t)
        return out

    def loadbias(name, n):
        """DRAM [n*128] vector -> [128, n] f32 column tile."""
        t = wp.tile([128, n], F32, name=f"{name}_b", tag=f"{name}_b")
        nc.sync.dma_start(t[:], aps[name].rearrange("(m p) -> p m", p=128))
        return t

    # ---- weights
    wq = loadw("wq", C, E)
    wk = loadw("wk", DD, E)
    wv = loadw("wv", DD, E)
    wkr = loadw("wkr", E, E)
    wvr = loadw("wvr", E, E)
    wst = loadw("wst", DS, E)
    wf1 = loadw("wf1", E, E)
    wf2 = loadw("wf2", E, C)
    wout = loadw("wout", E, E)
    wctx = loadw("wctx", E, C)
    maskt = loadw("maskt", S, H)                      # 8 x [128, 1024]
    mrow = wp.tile([1, H], BF)
    nc.sync.dma_start(mrow[:], aps["maskt"][S:S1, :])
    ident = wp.tile([128, 128], BF)
    nc.sync.dma_start(ident[:], aps["ident"])
    ones8 = wp.tile([1, NH], BF)
    nc.sync.dma_start(ones8[:], aps["ones8"])

    def loadbias64(name):
        t = wp.tile([64, NH], F32, name=f"{name}_b64", tag=f"{name}_b64")
        nc.sync.dma_start(t[:], aps[name].rearrange("(m p) -> p m", p=64))
        return t

    bq = loadbias64("bqv")
    bk = loadbias64("bkv")
    bkr = loadbias("bkrv", KE)
    bkr2 = loadbias64("bkrv")
    bst = loadbias("bstv", KE)
    bf1 = loadbias("bf1v", KE)
    bf2 = loadbias("bf2v", KC_C)
    cb = loadbias("cbv", KC_C)

    def bcast_row(name):
        t = wp.tile([128, E], F32, name=f"{name}_bc", tag=f"{name}_bc")
        src = aps[name]
        bb = bass.AP(tensor=src.tensor, offset=src.offset,
                     ap=[[0, 128]] + list(src.ap))
        nc.sync.dma_start(t[:], bb)
        return t

    bvbc = bcast_row("bvv")       # [128, 512] f32 broadcast rows
    bvrbc = bcast_row("bvrv")

    # ---- stage F: static token, film, k_static, v_static (both batches)
    stt = ab.tile([DS, BL], BF, tag="stt")
    nc.sync.dma_start(stt[:], aps["statt"])

    st = ab.tile([128, KE, BL], BF, tag="st")         # static_token^T chunks
    for m in range(KE):
        ps = pp_s.tile([128, 1024], F32, tag="ps")
        nc.tensor.matmul(ps[:, :BL], wst[0][:, m * 128:(m + 1) * 128], stt[:],
                         start=True, stop=True)
        nc.vector.tensor_scalar_add(st[:, m, :], ps[:, :BL], bst[:, m:m + 1])

    # film MLP -> scale[C, BL] = 1 + tanh(W2 @ leaky(W1 @ st + b1) + b2)
    h1 = ab.tile([128, KE, BL], BF, tag="h1")
    for m in range(KE):
        ps = pp_s.tile([128, 1024], F32, tag="ps")
        for kc in range(KE):
            nc.tensor.matmul(ps[:, :BL], wf1[kc][:, m * 128:(m + 1) * 128],
                             st[:, kc, :], start=(kc == 0), stop=(kc == KE - 1))
        t = ab.tile([128, BL], F32, tag="t")
        nc.vector.tensor_scalar_add(t[:], ps[:, :BL], bf1[:, m:m + 1])
        u = ab.tile([128, BL], F32, tag="u")
        nc.vector.tensor_scalar_mul(u[:], t[:], 0.1)
        nc.vector.tensor_max(h1[:, m, :], t[:], u[:])
    scale = ab.tile([128, KC_C, BL], F32, tag="scale")
    for m in range(KC_C):
        ps = pp_s.tile([128, 1024], F32, tag="ps")
        for kc in range(KE):
            nc.tensor.matmul(ps[:, :BL], wf2[kc][:, m * 128:(m + 1) * 128],
                             h1[:, kc, :], start=(kc == 0), stop=(kc == KE - 1))
        nc.scalar.activation(scale[:, m, :], ps[:, :BL], AF.Tanh,
                             bias=bf2[:, m:m + 1])
        nc.vector.tensor_scalar_add(scale[:, m, :], scale[:, m, :], 1.0)

    # k_static^T [E, BL] -> block-diag ks matrix per batch: [KE][128, NH]
    ksh = [[ab.tile([64, NH], BF, tag=f"ksh{b}_{hh}", name=f"ksh{b}_{hh}")
            for hh in range(NH)] for b in range(BL)]
    for b in range(BL):
        for hh in range(NH):
            nc.gpsimd.memset(ksh[b][hh][:], 0.0)
    for m in range(NH):
        ps = pp_s.tile([128, 1024], F32, tag="ps")
        for kc in range(KE):
            nc.tensor.matmul(ps[:64, :BL], wkr[kc][:, m * 64:(m + 1) * 64],
                             st[:, kc, :], start=(kc == 0), stop=(kc == KE - 1))
        uks = ab.tile([64, BL], BF, tag="uks")
        nc.vector.tensor_scalar_add(uks[:], ps[:64, :BL], bkr2[:, m:m + 1])
        for b in range(BL):
            nc.vector.tensor_copy(ksh[b][m][:, m:m + 1], uks[:, b:b + 1])

    # v_static rows with ones cols: v8[b] [1, NH*65]
    v8 = [ab.tile([1, NH * 65], BF, tag=f"v8_{b}", name=f"v8_{b}")
          for b in range(BL)]
    for b in range(BL):
        nc.sync.dma_start(v8[b][:], aps["onesv"][0:1, :])
        ps = pp_s.tile([128, 1024], F32, tag="ps")
        for kc in range(KE):
            nc.tensor.matmul(ps[0:1, :E], st[:, kc, b:b + 1], wvr[kc][:],
                             start=(kc == 0), stop=(kc == KE - 1))
        dst = v8[b][0:1, :].rearrange("p (h c) -> p h c", c=65)[:, :, 0:64]
        nc.vector.tensor_add(
            dst, ps[0:1, :E].rearrange("p (h c) -> p h c", c=64),
            bvrbc[0:1, :].rearrange("p (h c) -> p h c", c=64))

    # ---- per batch
    for b in range(BL):
        # stage A ------------------------------------------------------
        xbf = [ab.tile([128, H], BF, tag=f"x{m}", name=f"x{m}")
               for m in range(KC_C)]
        xf = [ab.tile([128, H], F32, tag=f"xf{m}", name=f"xf{m}")
              for m in range(KC_C)]
        for m in range(KC_C):
            nc.sync.dma_start(xbf[m][:], aps["x"][b, m * 128:(m + 1) * 128, :])
            nc.sync.dma_start(xf[m][:], aps["xf"][b, m * 128:(m + 1) * 128, :])
        dynt = [sc1.tile([128, S], BF, tag="sc1", name=f"dynt{kk}")
                for kk in range(KE)]
        for kc in range(KE):
            nc.sync.dma_start(dynt[kc][:],
                              aps["dynt"][b, kc * 128:(kc + 1) * 128, :])

        qt = [ab.tile([64, H], BF, tag=f"qt{m}", name=f"qt{m}")
              for m in range(NH)]
        kt = [ab.tile([64, S], BF, tag=f"kt{m}", name=f"kt{m}")
              for m in range(NH)]
        for m in range(NH):
            for n in range(NI):
                ns = slice(n * 512, (n + 1) * 512)
                ps = pp_s.tile([128, 1024], F32, tag="ps")
                for kc in range(KC_C):
                    nc.tensor.matmul(ps[:64, :512],
                                     wq[kc][:, m * 64:(m + 1) * 64],
                                     xbf[kc][:, ns], start=(kc == 0),
                                     stop=(kc == KC_C - 1))
                nc.vector.tensor_scalar_add(qt[m][:, ns], ps[:64, :512],
                                            bq[:, m:m + 1])
                ps2 = pp_s.tile([128, 1024], F32, tag="ps")
                for kc in range(KE):
                    nc.tensor.matmul(ps2[:64, :512],
                                     wk[kc][:, m * 64:(m + 1) * 64],
                                     dynt[kc][:, ns], start=(kc == 0),
                                     stop=(kc == KE - 1))
                nc.vector.tensor_scalar_add(kt[m][:, ns], ps2[:64, :512],
                                            bk[:, m:m + 1])

        v = [ab.tile([128, NH * 65], BF, tag=f"v{j}", name=f"v{j}")
             for j in range(NJ)]
        for j in range(NJ):
            nc.sync.dma_start(v[j][:], aps["onesv"])
            ps = pp_s.tile([128, 1024], F32, tag="ps")
            for kc in range(KE):
                nc.tensor.matmul(ps[:, :E], dynt[kc][:, j * 128:(j + 1) * 128],
                                 wv[kc][:], start=(kc == 0),
                                 stop=(kc == KE - 1))
            nc.vector.tensor_add(
                v[j][:].rearrange("p (h c) -> p h c", c=65)[:, :, 0:64],
                ps[:, :E].rearrange("p (h c) -> p h c", c=64),
                bvbc[:].rearrange("p (h c) -> p h c", c=64))

        # stage B ------------------------------------------------------
        # static-token scores for all heads at once: [NH, H]
        pst = pp_c.tile([NH, 1024], F32, tag="pc")
        for n in range(NI):
            ns = slice(n * 512, (n + 1) * 512)
            for hh in range(NH):
                nc.tensor.matmul(pst[:, ns], ksh[b][hh][:], qt[hh][:, ns],
                                 start=(hh == 0), stop=False)
            nc.tensor.matmul(pst[:, ns], ones8[:], mrow[:, ns],
                             start=False, stop=True)
        est = ab.tile([NH, H], BF, tag="est")
        nc.scalar.activation(est[:], pst[:], AF.Exp)
        est_d = dp.tile([NH, H], BF, name="est_d")
        nc.sync.dma_start(est_d[:], est[:])

        ctxt = [ab.tile([128, H], BF, tag=f"ctxt{m}", name=f"ctxt{m}")
                for m in range(KE)]
        for h in range(NH):
            pr, ho = h // 2, (h % 2) * 64
            erow = sp.tile([1, H], BF, tag="erow")
            nc.sync.dma_start(erow[:], est_d[h:h + 1, :])
            pc = pp_c.tile([65, 1024], F32, tag="pc")
            for j in range(NJ):
                its = (0, 1) if j < 4 else (1,)
                w = 512 * len(its)
                ps = pp_s.tile([128, 1024], F32, tag="ps")
                for ii, n in enumerate(its):
                    ns = slice(n * 512, (n + 1) * 512)
                    os_ = slice(ii * 512, ii * 512 + 512)
                    nc.tensor.matmul(ps[:, os_],
                                     kt[h][:, j * 128:(j + 1) * 128],
                                     qt[h][:, ns], start=True, stop=False)
                    nc.tensor.matmul(ps[:, os_], ident[:],
                                     maskt[j][:, ns], start=False, stop=True)
                e = ep.tile([128, 1024], BF, tag="e")
                nc.scalar.activation(e[:, :w], ps[:, :w], AF.Exp)
                for ii, n in enumerate(its):
                    ns = slice(n * 512, (n + 1) * 512)
                    os_ = slice(ii * 512, ii * 512 + 512)
                    nc.tensor.matmul(pc[:, ns],
                                     v[j][:, h * 65:(h + 1) * 65],
                                     e[:, os_], start=(j == 0), stop=False)
            # static-token ctx contribution closes each accumulation group
            for n in range(NI):
                ns = slice(n * 512, (n + 1) * 512)
                nc.tensor.matmul(pc[:, ns],
                                 v8[b][0:1, h * 65:(h + 1) * 65],
                                 erow[0:1, ns], start=False, stop=True)
            # evacuate PSUM fast so the next head's ctx can start
            cu = sp.tile([65, H], F32, tag="cu")
            nc.vector.tensor_copy(cu[:], pc[:])
            # reciprocal of denominator row (DVE approx); bcast via DRAM
            rec = sp.tile([65, H], F32, tag="rec")
            nc.vector.reciprocal_approx_fast(rec[64:65, :], cu[64:65, :])
            rd = dp.tile([1, H], F32)
            nc.sync.dma_start(rd[:], rec[64:65, :])
            rb = rp.tile([64, H], F32, tag="rb")
            rsrc = rd[:]
            nc.sync.dma_start(rb[:], bass.AP(tensor=rsrc.tensor,
                                             offset=rsrc.offset,
                                             ap=[[0, 64]] + list(rsrc.ap)[1:]))
            if ho == 0:
                nc.vector.tensor_mul(ctxt[pr][0:64, :], cu[0:64, :], rb[:])
            else:
                cm = rp.tile([64, H], BF, tag="cm")
                nc.vector.tensor_mul(cm[:], cu[0:64, :], rb[:])
                nc.sync.dma_start(ctxt[pr][64:128, :], cm[:])

        # stage C ------------------------------------------------------
        outt = [sc1.tile([128, H], BF, tag="sc1", name=f"outt{kk}")
                for kk in range(KE)]
        for m in range(KE):
            for n in range(NI):
                ns = slice(n * 512, (n + 1) * 512)
                ps = pp_s.tile([128, 1024], F32, tag="ps")
                for kc in range(KE):
                    nc.tensor.matmul(ps[:, :512],
                                     wout[kc][:, m * 128:(m + 1) * 128],
                                     ctxt[kc][:, ns], start=(kc == 0),
                                     stop=(kc == KE - 1))
                nc.scalar.copy(outt[m][:, ns], ps[:, :512])
        for m in range(KC_C):
            o = op.tile([128, H], F32, tag="o")
            for n in range(NI):
                ns = slice(n * 512, (n + 1) * 512)
                ps = pp_s.tile([128, 1024], F32, tag="ps")
                for kc in range(KE):
                    nc.tensor.matmul(ps[:, :512],
                                     wctx[kc][:, m * 128:(m + 1) * 128],
                                     outt[kc][:, ns], start=(kc == 0),
                                     stop=(kc == KE - 1))
                t = op.tile([128, 512], F32, tag="tt")
                nc.vector.tensor_add(t[:], ps[:, :512], xf[m][:, ns])
                nc.vector.tensor_scalar(o[:, ns], t[:], cb[:, m:m + 1],
                                        scale[:, m, b:b + 1],
                                        mybir.AluOpType.add,
                                        mybir.AluOpType.mult)
            nc.sync.dma_start(aps["out"][b, m * 128:(m + 1) * 128, :], o[:])


_CACHE = {}


def _build():
    if "nc" in _CACHE:
        return _CACHE["nc"]
    import concourse.tile as tile
    from concourse import bacc, mybir

    F32 = mybir.dt.float32
    BF = mybir.dt.bfloat16
    nc = bacc.Bacc("TRN2", target_bir_lowering=False, debug=False)
    specs = [
        ("x", [BL, C, H], BF), ("xf", [BL, C, H], F32),
        ("dynt", [BL, DD, S], BF), ("statt", [DS, BL], BF),
        ("wq", [C, E], BF), ("wk", [DD, E], BF), ("wv", [DD, E], BF),
        ("wkr", [E, E], BF), ("wvr", [E, E], BF), ("wst", [DS, E], BF),
        ("wf1", [E, E], BF), ("wf2", [E, C], BF),
        ("wout", [E, E], BF), ("wctx", [E, C], BF),
        ("maskt", [S1, H], BF), ("ident", [128, 128], BF),
        ("onesv", [128, NH * 65], BF),
        ("ones8", [1, NH], BF),
        ("bqv", [E], F32), ("bkv", [E], F32), ("bvv", [E], F32),
        ("bkrv", [E], F32), ("bvrv", [E], F32), ("bstv", [E], F32),
        ("bf1v", [E], F32), ("bf2v", [C], F32), ("cbv", [C], F32),
    ]
    aps = {}
    for name, shape, dt in specs:
        aps[name] = nc.dram_tensor(name, shape, dt, kind="ExternalInput").ap()
    aps["out"] = nc.dram_tensor("out", [BL, C, H], F32,
                                kind="ExternalOutput").ap()
    with tile.TileContext(nc) as tc, ExitStack() as ctx:
        _emit(ctx, tc, aps)
    nc.compile()
    _CACHE["nc"] = nc
    return nc


def kernel(**inputs) -> np.ndarray:
    from concourse.bass_utils import run_bass_kernel_spmd

    nc = _build()
    per_core = _host_prep(inputs)
    trace = bool(int(os.environ.get("KBENCH_TRACE", "0")))
    res = run_bass_kernel_spmd(nc, per_core, list(range(NCORES)), trace=trace)
    if trace and res.exec_time_ns is not None:
        print(f"HW exec time: {res.exec_time_ns} ns")
        _CACHE["exec_time_ns"] = res.exec_time_ns
        _CACHE["trace"] = res.instructions_and_trace
    out = np.concatenate([res.results[i]["out"] for i in range(NCORES)], 0)
    return np.ascontiguousarray(out, dtype=np.float32)
